# revision 38
# baseline (speedup 1.0000x reference)
"""Causal multi-head attention (B=2, S=2048, D=1024, H=16, hd=64) on 8 trn2 cores.

Sharding: core c handles batch b = c//4 and head group hg = c%4 (4 heads each).
Each core computes its Q/K/V shard (tensor-parallel columns of W_qkv), causal
attention for its 4 heads with scores held transposed ([s_k, s_q]), and a
partial output projection over its 256 rows of W_proj. The host sums the 4
partials per batch and adds the exact bias terms (softmax rows sum to 1, so
attn@(V + 1 bv^T) = attn@V + bv^T; the b_qkv V-slice and b_proj are applied on
the host).

PE packing (the key win over the naive schedule): the systolic array is
addressed in 32-row/32-col tiles, so matmuls on disjoint row/col groups run
concurrently when adjacent in the queue:
  - scores: K=64 per head -> the two heads of a pair run row-tiled at
    tile positions (0,0)/(64,0), one [128,1024] psum tile (A|B layout, one
    bank each), ~217ns per 512-q-col pair instead of 2x216.
  - PV: M=64 per head -> col-tiled pair at (0,0)/(0,64) into one [128,512]
    psum tile (head A partitions 0-63, head B 64-127), separate moving
    operands (the A|B halves of the P tile).
  - softmax denominators: 4 heads' ones-row matmuls (M=1) col-tiled at
    (0,{0,32,64,96}) into one psum bank -> all 4 denominators per 512 cols
    in ~227ns (replaces the 65th-V-column trick, freeing PV for col tiling).
Pairs share one psum tile so both MMs have identical WAR deps and the
priority scheduler keeps them adjacent (separated pairs don't overlap).

Attention runs on 512-col q blocks (4 per core), all 4 heads per iteration
over ki; pv/den lag scores by 2 iterations to hide the exp latency. Each
128-col q chunk stops accumulating at its diagonal ki, so finish (reciprocal
-> gpsimd broadcast -> scale, two-phase) and the output projection stagger
through the whole kernel instead of trailing it.

DMA: all input DMAs ride the sync issue queue in strict priority order (wq,
x cols 0:512, bqk, mask, wk, wv, x 512:1024, wp, rest of x); the 16 DMA ring
engines drain descriptors in arrival order, so issue order ~= completion
order and the critical prefix gets full HBM bandwidth. Junk warmup matmuls
run during the DMA wait (and in the thin drain iterations at the end) so the
PE's HAM clock gate stays released while real work is pending.
"""

import numpy as np
import ml_dtypes
from contextlib import ExitStack

B, S, D, H = 2, 2048, 1024, 16
HD = 64
NCORES = 8
FPC = 256  # features per core (4 heads x 64)

_CACHE = {}


def _build():
    import concourse.bacc as bacc
    import concourse.tile as tile
    import concourse.mybir as mybir

    f32 = mybir.dt.float32
    bf16 = mybir.dt.bfloat16

    nc = bacc.Bacc("TRN2", target_bir_lowering=False, debug=False, num_devices=NCORES)

    xT = nc.dram_tensor("xT", [D, S], bf16, kind="ExternalInput").ap()
    wq = nc.dram_tensor("wq", [D, FPC], bf16, kind="ExternalInput").ap()
    wk = nc.dram_tensor("wk", [D, FPC], bf16, kind="ExternalInput").ap()
    wv = nc.dram_tensor("wv", [D, FPC], bf16, kind="ExternalInput").ap()
    wp = nc.dram_tensor("wp", [FPC, D], bf16, kind="ExternalInput").ap()
    bqk = nc.dram_tensor("bqk", [128, 4], f32, kind="ExternalInput").ap()
    maskT = nc.dram_tensor("maskT", [128, 256], bf16, kind="ExternalInput").ap()
    out = nc.dram_tensor("out", [S, D], bf16, kind="ExternalOutput").ap()

    with tile.TileContext(nc) as tc:
        with ExitStack() as ctx:
            _body(ctx, tc, mybir, out, xT, wq, wk, wv, wp, bqk, maskT)

    nc.compile()
    return nc


def _body(ctx, tc, mybir, out, xT, wq, wk, wv, wp, bqk, maskT):
    from functools import partial

    nc = tc.nc
    f32 = mybir.dt.float32
    bf16 = mybir.dt.bfloat16
    AF = mybir.ActivationFunctionType
    NK = D // 128   # 8 contraction tiles for qkv/proj-input dim
    NS = S // 128   # 16 sequence (key) tiles
    LAG = 2         # iterations between scores and pv/den
    SCL = float(HD) ** -0.5

    sb = ctx.enter_context(tc.tile_pool(name="sb", bufs=1))

    xt_a = sb.tile([128, NK * S], bf16, name="xta", tag="xta")
    xt_t = [xt_a[:, k * S:(k + 1) * S] for k in range(NK)]
    wq_a = sb.tile([128, NK * FPC], bf16, name="wqa", tag="wqa")
    wq_t = [wq_a[:, k * FPC:(k + 1) * FPC] for k in range(NK)]
    wk_a = sb.tile([128, NK * FPC], bf16, name="wka", tag="wka")
    wk_t = [wk_a[:, k * FPC:(k + 1) * FPC] for k in range(NK)]
    wv_a = sb.tile([128, NK * FPC], bf16, name="wva", tag="wva")
    wv_t = [wv_a[:, k * FPC:(k + 1) * FPC] for k in range(NK)]
    wp_a = sb.tile([128, 2 * D], bf16, name="wpa", tag="wpa")
    wp_t = [wp_a[:, k * D:(k + 1) * D] for k in range(2)]
    qt_t = [sb.tile([128, S], bf16, name=f"qtt{f}", tag=f"qtt{f}") for f in range(2)]
    kt_t = [sb.tile([128, S], bf16, name=f"ktt{f}", tag=f"ktt{f}") for f in range(2)]
    v_t = [sb.tile([128, FPC], bf16, name=f"vt{s}", tag=f"vt{s}") for s in range(NS)]
    ot_t = [sb.tile([128, S], bf16, name=f"ott{f}", tag=f"ott{f}") for f in range(2)]
    bqk_t = sb.tile([128, 4], f32, name="bqkt", tag="bqkt")
    mask_t = sb.tile([128, 256], bf16, name="maskt", tag="maskt")
    ones_t = sb.tile([128, 1], bf16, name="ones1", tag="ones1")
    wu_t = sb.tile([128, 512], bf16, name="wut", tag="wut")
    nc.vector.memset(ones_t[:], 1.0)
    nc.vector.memset(wu_t[:], 0.0)

    # ---- input DMAs, ALL on the sync issue queue in strict priority order:
    # the 16 DMA ring engines serve descriptors in arrival order, so a single
    # ordered stream gives the critical prefix (wq, x cols 0:512) the full
    # HBM bandwidth instead of racing later tensors.
    xt3 = xt_a.rearrange("p (k s) -> p k s", k=NK)
    xs3 = xT.rearrange("(k p) s -> p k s", p=128)
    nc.sync.dma_start(wq_a.rearrange("p (k f) -> p k f", k=NK),
                      wq.rearrange("(k p) f -> p k f", p=128))
    nc.sync.dma_start(xt3[:, 0:4, 0:512], xs3[:, 0:4, 0:512])
    nc.sync.dma_start(xt3[:, 4:8, 0:512], xs3[:, 4:8, 0:512])
    nc.sync.dma_start(bqk_t[:], bqk[:])
    nc.sync.dma_start(mask_t[:], maskT[:])
    nc.sync.dma_start(wk_a.rearrange("p (k f) -> p k f", k=NK),
                      wk.rearrange("(k p) f -> p k f", p=128))
    nc.sync.dma_start(wv_a.rearrange("p (k f) -> p k f", k=NK),
                      wv.rearrange("(k p) f -> p k f", p=128))
    nc.sync.dma_start(xt3[:, :, 512:1024], xs3[:, :, 512:1024])
    nc.sync.dma_start(wp_a.rearrange("p (k f) -> p k f", k=2),
                      wp.rearrange("(k p) f -> p k f", p=128))
    nc.sync.dma_start(xt3[:, :, 1024:1536], xs3[:, :, 1024:1536])
    nc.sync.dma_start(xt3[:, :, 1536:2048], xs3[:, :, 1536:2048])

    # PSUM (8 banks): scores 2x[128,1024] (4), pv 2x[128,512] (2),
    # den 1x[128,512] (1), filler ring 1x[128,512] (1).
    scp = ctx.enter_context(tc.tile_pool(name="ps_sc", bufs=2, space="PSUM"))
    pvp = ctx.enter_context(tc.tile_pool(name="ps_pv", bufs=2, space="PSUM"))
    dnp = ctx.enter_context(tc.tile_pool(name="ps_dn", bufs=1, space="PSUM"))
    flp = ctx.enter_context(tc.tile_pool(name="ps_fl", bufs=1, space="PSUM"))

    p_pool = ctx.enter_context(tc.tile_pool(name="pp", bufs=1))
    rc_pool = ctx.enter_context(tc.tile_pool(name="rcp", bufs=8))
    oo_pool = ctx.enter_context(tc.tile_pool(name="oop", bufs=3))

    # ---- warmup: junk matmuls so HAM releases the PE clock gate before the
    # first real matmul; runs entirely during the DMA wait.
    wups = flp.tile([128, 512], f32, name="fl", tag="fl", bufs=1)
    for i in range(16):
        nc.tensor.matmul(wups[:], wu_t[:, 0:128], wu_t[:], start=True,
                         stop=True, skip_group_check=True)

    def qkt_half(dst, w_t, bcol, f, c, pool=None):
        """512-col chunk of the Q^T/K^T projection for feature split f.
        Prelude groups allocate from the (still idle) scores ring so they
        double-buffer; in-block fillers use the single filler bank."""
        tg = "sc" if pool is not None else "fl"
        ps = (pool or flp).tile([128, 512], f32, name=tg, tag=tg,
                                bufs=2 if pool is not None else 1)
        for k in range(NK):
            nc.tensor.matmul(
                ps[:],
                w_t[k][:, f * 128:(f + 1) * 128],
                xt_t[k][:, c * 512:(c + 1) * 512],
                start=(k == 0), stop=(k == NK - 1),
            )
        nc.vector.tensor_scalar_add(
            dst[f][:, c * 512:(c + 1) * 512], ps[:],
            bqk_t[:, bcol + f: bcol + f + 1],
        )

    def v_group(s, pool=None):
        tg = "sc" if pool is not None else "fl"
        psv = (pool or flp).tile([128, FPC], f32, name=tg, tag=tg,
                                 bufs=2 if pool is not None else 1)
        for k in range(NK):
            nc.tensor.matmul(
                psv[:],
                xt_t[k][:, s * 128:(s + 1) * 128],
                wv_t[k][:],
                start=(k == 0), stop=(k == NK - 1),
            )
        nc.vector.tensor_copy(v_t[s][:], psv[:])

    def proj_half(s, nh, act_drain=False, pool=None):
        """512 output columns of the projection for q rows [128s,128s+128)."""
        tg = "sc" if pool is not None else "fl"
        pj = (pool or flp).tile([128, 512], f32, name=tg, tag=tg,
                                bufs=2 if pool is not None else 1)
        for k2 in range(2):
            nc.tensor.matmul(
                pj[:],
                ot_t[k2][:, s * 128:(s + 1) * 128],
                wp_t[k2][:, nh * 512:(nh + 1) * 512],
                start=(k2 == 0), stop=(k2 == 1),
            )
        oo = oo_pool.tile([128, 512], bf16, name="oo", tag="oo", bufs=3)
        if act_drain:
            nc.scalar.copy(oo[:], pj[:])
        else:
            nc.vector.tensor_copy(oo[:], pj[:])
        nc.sync.dma_start(out[s * 128:(s + 1) * 128, nh * 512:(nh + 1) * 512],
                          oo[:])

    class Block:
        """Causal attention for all 4 heads over q cols [512b, 512b+512)."""

        def __init__(self, b):
            self.b = b
            self.n = 4 * b + 4          # number of ki blocks
            self.q0 = 512 * b
            self.pv = [pvp.tile([128, 512], f32, name="pv", tag="pv", bufs=2)
                       for _ in range(2)]
            self.dn = dnp.tile([128, 512], f32, name="dn", tag="dn", bufs=1)
            self.P = {}
            self.a0s = {}

        def a0(self, ki):
            return 128 * (ki - 4 * self.b) if ki >= 4 * self.b else 0

        def scores(self, ki):
            """Row-tiled pair per hp: heads 2hp (rows 0:64 -> cols 0:512 of
            sc) and 2hp+1 (rows 64:128 -> cols 512:1024)."""
            a0 = self.a0(ki)
            diag = ki >= 4 * self.b
            for hp in range(2):
                sc = scp.tile([128, 1024], f32, name="sc", tag="sc", bufs=2)
                qt, kt = qt_t[hp], kt_t[hp]
                for r0, off in ((0, 0), (64, 512)):
                    nc.tensor.matmul(
                        sc[:, off + a0:off + 512],
                        kt[r0:r0 + 64, ki * 128:(ki + 1) * 128],
                        qt[r0:r0 + 64, self.q0 + a0:self.q0 + 512],
                        start=True, stop=True,
                    )
                P = p_pool.tile([128, 1024], bf16, name="P", tag="P", bufs=8)
                if a0:
                    # one strided instr covering both heads' valid spans
                    sc3 = sc.rearrange("p (h c) -> p h c", h=2)
                    P3 = P.rearrange("p (h c) -> p h c", h=2)
                    nc.scalar.activation(P3[:, :, a0:512], sc3[:, :, a0:512],
                                         AF.Exp, scale=SCL)
                else:
                    nc.scalar.activation(P[:], sc[:], AF.Exp, scale=SCL)
                if diag:
                    P3 = P.rearrange("p (h c) -> p h c", h=2)
                    m3 = mask_t.rearrange("p (h c) -> p h c", h=2)
                    nc.vector.tensor_mul(P3[:, :, a0:a0 + 128],
                                         P3[:, :, a0:a0 + 128], m3[:])
                self.P[(hp, ki)] = P

        def pv_den(self, ki):
            """Col-tiled PV pair per hp + 4-wide den pass; 128-col chunk
            (ki-4b) stops at its diagonal ki."""
            a0 = self.a0(ki)
            diag = ki >= 4 * self.b
            spans = [(a0, a0 + 128, True)] if diag else []
            if diag and a0 + 128 < 512:
                spans.append((a0 + 128, 512, False))
            if not diag:
                spans = [(0, 512, False)]
            Ps = [self.P.pop((0, ki)), self.P.pop((1, ki))]
            for a, bnd, stops in spans:
                for hp in range(2):
                    P = Ps[hp]
                    for r0, off in ((0, 0), (64, 512)):
                        hcol = (2 * hp + (1 if r0 else 0)) * 64
                        nc.tensor.matmul(
                            self.pv[hp][r0:r0 + 64, a:bnd],
                            v_t[ki][:, hcol:hcol + 64],
                            P[:, off + a:off + bnd],
                            start=(ki == 0), stop=stops,
                        )
                for j in range(4):
                    P, off = Ps[j // 2], 512 * (j % 2)
                    nc.tensor.matmul(
                        self.dn[32 * j:32 * j + 1, a:bnd],
                        ones_t[:],
                        P[:, off + a:off + bnd],
                        start=(ki == 0), stop=stops,
                        tile_position=(0, 32 * j),
                    )

        def finish_a(self, a, w):
            """Cols [a,a+w) released: copy each denominator row out of psum
            (the custom reciprocal op misreads psum at partition base 32/96,
            plain copies do not — DVE proven at base 0/32, ACT at 64/96,
            which also splits the burst across two engines), reciprocal,
            gpsimd partition broadcast. finish_b consumes the rbc tiles one
            iteration later."""
            rbcs = []
            for h in range(4):
                dcp = rc_pool.tile([1, 512], f32, name="dc", tag="dc", bufs=8)
                if h < 2:
                    nc.vector.tensor_copy(
                        dcp[:, 0:w], self.dn[32 * h:32 * h + 1, a:a + w])
                else:
                    nc.scalar.copy(
                        dcp[:, 0:w], self.dn[32 * h:32 * h + 1, a:a + w])
                rcp = rc_pool.tile([1, 512], f32, name="rc", tag="rc", bufs=8)
                nc.vector.reciprocal_approx_fast(rcp[:, 0:w], dcp[:, 0:w])
                rbc = rc_pool.tile([64, 512], f32, name="rb", tag="rb", bufs=8)
                nc.gpsimd.partition_broadcast(rbc[:, 0:w], rcp[:, 0:w],
                                              channels=64)
                rbcs.append(rbc)
            return rbcs

        def finish_b(self, a, w, rbcs):
            for hp in range(2):
                for r0 in (0, 64):
                    h = 2 * hp + (1 if r0 else 0)
                    nc.vector.tensor_mul(
                        ot_t[hp][r0:r0 + 64, self.q0 + a:self.q0 + a + w],
                        self.pv[hp][r0:r0 + 64, a:a + w],
                        rbcs[h][:, 0:w])

    def warm_fill(n):
        """Keep-warm junk matmuls for thin drain iterations (sc ring is free
        once the last scores have issued)."""
        dum = scp.tile([128, 512], f32, name="sc", tag="sc", bufs=2)
        for _ in range(n):
            nc.tensor.matmul(dum[:], wu_t[:, 0:128], wu_t[:], start=True,
                             stop=True, skip_group_check=True)

    # ---- prelude: q/k projections for chunk 0, both feature splits. These
    # MUST be emitted before the first scores — the tile dep tracker links a
    # read only to writes emitted before it, so a later-emitted producer is
    # silently unordered (reads garbage). Prelude groups allocate from the
    # still-idle scores ring so consecutive groups double-buffer.
    qkt_half(qt_t, wq_t, 0, 0, 0, pool=scp)
    qkt_half(kt_t, wk_t, 2, 0, 0, pool=scp)
    qkt_half(qt_t, wq_t, 0, 1, 0, pool=scp)
    qkt_half(kt_t, wk_t, 2, 1, 0, pool=scp)

    # ---- flat global pipeline: the scores stream runs contiguously across
    # the four q blocks (no per-block drain bubbles); pv/den trail by LAG.
    # Block b's pool tiles are recycled exactly when its coarse finish (read)
    # precedes the next block's first accumulation (write) inside one
    # iteration. Fillers are placed per global iteration g.
    P = partial
    SL = [(b, ki) for b in range(4) for ki in range(4 * b + 4)]
    fillers = {
        0: [P(v_group, 0)],
        1: [P(v_group, 1)],
        2: [P(qkt_half, qt_t, wq_t, 0, 0, 1), P(v_group, 2)],
        3: [P(qkt_half, qt_t, wq_t, 0, 1, 1), P(v_group, 3)],
        4: [P(qkt_half, kt_t, wk_t, 2, 0, 1)],
        5: [P(qkt_half, kt_t, wk_t, 2, 1, 1)],
        6: [P(v_group, 4)],
        7: [P(v_group, 5)],
        8: [P(qkt_half, qt_t, wq_t, 0, 0, 2), P(v_group, 6)],
        9: [P(qkt_half, qt_t, wq_t, 0, 1, 2), P(v_group, 7)],
        10: [P(qkt_half, kt_t, wk_t, 2, 0, 2)],
        11: [P(qkt_half, kt_t, wk_t, 2, 1, 2)],
        12: [P(v_group, 8)],
        13: [P(v_group, 9)],
        14: [P(v_group, 10)],
        15: [P(v_group, 11)],
        16: [P(qkt_half, qt_t, wq_t, 0, 0, 3)],
        17: [P(qkt_half, qt_t, wq_t, 0, 1, 3)],
        18: [P(proj_half, 0, 0)],
        19: [P(proj_half, 0, 1)],
        20: [P(proj_half, 1, 0)],
        21: [P(proj_half, 1, 1)],
        22: [P(proj_half, 2, 0)],
        23: [P(proj_half, 2, 1)],
        24: [P(v_group, 12)],
        25: [P(v_group, 13)],
        26: [P(v_group, 14)],
        27: [P(v_group, 15)],
        28: [P(proj_half, 3, 0)],
        29: [P(proj_half, 3, 1)],
        30: [P(qkt_half, kt_t, wk_t, 2, 0, 3)],
        31: [P(qkt_half, kt_t, wk_t, 2, 1, 3)],
        32: [P(proj_half, 4, 0)],
        33: [P(proj_half, 4, 1)],
        34: [P(proj_half, 5, 0)],
        35: [P(proj_half, 5, 1)],
        36: [P(proj_half, 6, 0), P(proj_half, 6, 1)],
        37: [P(proj_half, 7, 0), P(proj_half, 7, 1)],
        38: [P(proj_half, 8, 0, True), P(proj_half, 8, 1, False),
             P(warm_fill, 2)],
        39: [P(proj_half, 9, 0, True), P(proj_half, 9, 1, False),
             P(warm_fill, 2)],
        40: [P(proj_half, 10, 0, True, scp), P(proj_half, 10, 1, False, scp),
             P(proj_half, 12, 0, True, scp), P(proj_half, 12, 1, False, scp),
             P(warm_fill, 6)],
        41: [P(proj_half, 11, 0, True, scp), P(proj_half, 11, 1, False, scp),
             P(proj_half, 13, 0, True, scp), P(proj_half, 13, 1, False, scp),
             P(warm_fill, 6)],
    }
    blocks = {}
    pend = []
    for g in range(len(SL) + LAG):
        for fb in pend:
            fb()
        pend = []
        if g < len(SL):
            b, ki = SL[g]
            if ki == 0:
                blocks[b] = Block(b)
            blocks[b].scores(ki)
        if g >= LAG:
            b, kj = SL[g - LAG]
            blk = blocks[b]
            blk.pv_den(kj)
            if b == 3 and kj >= 12:
                a = 128 * (kj - 12)
                pend.append(
                    partial(blk.finish_b, a, 128, blk.finish_a(a, 128)))
            elif b < 3 and kj >= 4 * b + 1:
                # (0,256) @ diag+1, (256,384) @ diag+2, (384,512) @ diag+3:
                # the final part is 128 cols, so the next block's first pv
                # accumulation (WAR on these reads) clears quickly.
                a, w = {1: (0, 256), 2: (256, 128), 3: (384, 128)}[kj - 4 * b]
                pend.append(
                    partial(blk.finish_b, a, w, blk.finish_a(a, w)))
        for fn in fillers.get(g, []):
            fn()
    warm_fill(3)
    for fb in pend:
        fb()
    proj_half(14, 0, True, pool=scp)
    proj_half(14, 1, False, pool=scp)
    warm_fill(3)
    proj_half(15, 0, True, pool=scp)
    proj_half(15, 1, False, pool=scp)


def _in_maps(x, W_qkv, b_qkv, W_proj):
    bf = ml_dtypes.bfloat16
    maps = []
    # multiplicative causal mask for the transposed diag block: keep k<=q,
    # two copies side by side so one strided multiply covers a head pair
    mask1 = np.triu(np.ones((128, 128), np.float32))
    mask = np.concatenate([mask1, mask1], axis=1).astype(bf)
    for core in range(NCORES):
        b, hg = core // 4, core % 4
        cs = slice(hg * FPC, (hg + 1) * FPC)
        bq = b_qkv[cs].astype(np.float32)
        bk = b_qkv[D + hg * FPC: D + (hg + 1) * FPC].astype(np.float32)
        maps.append({
            "xT": np.ascontiguousarray(x[b].T).astype(bf),
            "wq": np.ascontiguousarray(W_qkv[:, cs]).astype(bf),
            "wk": np.ascontiguousarray(W_qkv[:, D + hg * FPC: D + (hg + 1) * FPC]).astype(bf),
            "wv": np.ascontiguousarray(W_qkv[:, 2 * D + hg * FPC: 2 * D + (hg + 1) * FPC]).astype(bf),
            "wp": np.ascontiguousarray(W_proj[hg * FPC:(hg + 1) * FPC, :]).astype(bf),
            "bqk": np.ascontiguousarray(
                np.stack([bq[0:128], bq[128:256], bk[0:128], bk[128:256]], axis=1)),
            "maskT": mask,
        })
    return maps


def get_nc():
    if "nc" not in _CACHE:
        _CACHE["nc"] = _build()
    return _CACHE["nc"]


def _postprocess(partials, b_qkv, W_proj, b_proj):
    out = np.zeros((B, S, D), np.float32)
    for core in range(NCORES):
        out[core // 4] += np.asarray(partials[core], np.float32)
    bv = np.asarray(b_qkv, np.float32)[2 * D:3 * D]
    out += bv @ np.asarray(W_proj, np.float32) + np.asarray(b_proj, np.float32)
    return out


def kernel(x, W_qkv, b_qkv, W_proj, b_proj, _trace=False):
    from concourse.bass_utils import run_bass_kernel_spmd

    x = np.asarray(x, np.float32)
    W_qkv = np.asarray(W_qkv, np.float32)
    b_qkv = np.asarray(b_qkv, np.float32)
    W_proj = np.asarray(W_proj, np.float32)
    b_proj = np.asarray(b_proj, np.float32)

    nc = get_nc()
    maps = _in_maps(x, W_qkv, b_qkv, W_proj)
    res = run_bass_kernel_spmd(nc, maps, list(range(NCORES)), trace=_trace)
    _CACHE["last_result"] = res
    partials = [res.results[c]["out"] for c in range(NCORES)]
    return _postprocess(partials, b_qkv, W_proj, b_proj)


# revision 39
# speedup vs baseline: 1.1208x; 1.1208x over previous
"""Causal multi-head attention (B=2, S=2048, D=1024, H=16, hd=64) on 8 trn2 cores.

Sharding: core c handles batch b = c//4 and head group hg = c%4 (4 heads each).
Each core computes its Q/K/V shard (tensor-parallel columns of W_qkv), causal
attention for its 4 heads with scores held transposed ([s_k, s_q]), and a
partial output projection over its 256 rows of W_proj. The host sums the 4
partials per batch and adds the exact bias terms (softmax rows sum to 1, so
attn@(V + 1 bv^T) = attn@V + bv^T; the b_qkv V-slice and b_proj are applied on
the host).

PE packing (the key win over the naive schedule): the systolic array is
addressed in 32-row/32-col tiles, so matmuls on disjoint row/col groups run
concurrently when adjacent in the queue:
  - scores: K=64 per head -> the two heads of a pair run row-tiled at
    tile positions (0,0)/(64,0), one [128,1024] psum tile (A|B layout, one
    bank each), ~217ns per 512-q-col pair instead of 2x216.
  - PV: M=64 per head -> col-tiled pair at (0,0)/(0,64) into one [128,512]
    psum tile (head A partitions 0-63, head B 64-127), separate moving
    operands (the A|B halves of the P tile).
  - softmax denominators: 4 heads' ones-row matmuls (M=1) col-tiled at
    (0,{0,32,64,96}) into one psum bank -> all 4 denominators per 512 cols
    in ~227ns (replaces the 65th-V-column trick, freeing PV for col tiling).
Pairs share one psum tile so both MMs have identical WAR deps and the
priority scheduler keeps them adjacent (separated pairs don't overlap).

Attention runs on 512-col q blocks (4 per core), all 4 heads per iteration
over ki; pv/den lag scores by 2 iterations to hide the exp latency. Each
128-col q chunk stops accumulating at its diagonal ki, so finish (reciprocal
-> gpsimd broadcast -> scale, two-phase) and the output projection stagger
through the whole kernel instead of trailing it.

DMA: all input DMAs ride the sync issue queue in strict priority order (wq,
x cols 0:512, bqk, mask, wk, wv, x 512:1024, wp, rest of x); the 16 DMA ring
engines drain descriptors in arrival order, so issue order ~= completion
order and the critical prefix gets full HBM bandwidth. Junk warmup matmuls
run during the DMA wait (and in the thin drain iterations at the end) so the
PE's HAM clock gate stays released while real work is pending.
"""

import numpy as np
import ml_dtypes
from contextlib import ExitStack

B, S, D, H = 2, 2048, 1024, 16
HD = 64
NCORES = 8
FPC = 256  # features per core (4 heads x 64)

_CACHE = {}


def _build():
    import concourse.bacc as bacc
    import concourse.tile as tile
    import concourse.mybir as mybir

    f32 = mybir.dt.float32
    bf16 = mybir.dt.bfloat16

    nc = bacc.Bacc("TRN2", target_bir_lowering=False, debug=False, num_devices=NCORES)

    xT = nc.dram_tensor("xT", [D, S], bf16, kind="ExternalInput").ap()
    wq = nc.dram_tensor("wq", [D, FPC], bf16, kind="ExternalInput").ap()
    wk = nc.dram_tensor("wk", [D, FPC], bf16, kind="ExternalInput").ap()
    wv = nc.dram_tensor("wv", [D, FPC], bf16, kind="ExternalInput").ap()
    wp = nc.dram_tensor("wp", [FPC, D], bf16, kind="ExternalInput").ap()
    bqk = nc.dram_tensor("bqk", [128, 4], f32, kind="ExternalInput").ap()
    maskT = nc.dram_tensor("maskT", [128, 256], bf16, kind="ExternalInput").ap()
    out = nc.dram_tensor("out", [S, D], bf16, kind="ExternalOutput").ap()

    with tile.TileContext(nc) as tc:
        with ExitStack() as ctx:
            _body(ctx, tc, mybir, out, xT, wq, wk, wv, wp, bqk, maskT)

    nc.compile()
    return nc


def _body(ctx, tc, mybir, out, xT, wq, wk, wv, wp, bqk, maskT):
    from functools import partial

    nc = tc.nc
    f32 = mybir.dt.float32
    bf16 = mybir.dt.bfloat16
    AF = mybir.ActivationFunctionType
    NK = D // 128   # 8 contraction tiles for qkv/proj-input dim
    NS = S // 128   # 16 sequence (key) tiles
    LAG = 2         # iterations between scores and pv/den
    SCL = float(HD) ** -0.5

    sb = ctx.enter_context(tc.tile_pool(name="sb", bufs=1))

    xt_a = sb.tile([128, NK * S], bf16, name="xta", tag="xta")
    xt_t = [xt_a[:, k * S:(k + 1) * S] for k in range(NK)]
    wq_a = sb.tile([128, NK * FPC], bf16, name="wqa", tag="wqa")
    wq_t = [wq_a[:, k * FPC:(k + 1) * FPC] for k in range(NK)]
    wk_a = sb.tile([128, NK * FPC], bf16, name="wka", tag="wka")
    wk_t = [wk_a[:, k * FPC:(k + 1) * FPC] for k in range(NK)]
    wv_a = sb.tile([128, NK * FPC], bf16, name="wva", tag="wva")
    wv_t = [wv_a[:, k * FPC:(k + 1) * FPC] for k in range(NK)]
    wp_a = sb.tile([128, 2 * D], bf16, name="wpa", tag="wpa")
    wp_t = [wp_a[:, k * D:(k + 1) * D] for k in range(2)]
    qt_t = [sb.tile([128, S], bf16, name=f"qtt{f}", tag=f"qtt{f}") for f in range(2)]
    kt_t = [sb.tile([128, S], bf16, name=f"ktt{f}", tag=f"ktt{f}") for f in range(2)]
    v_t = [sb.tile([128, FPC], bf16, name=f"vt{s}", tag=f"vt{s}") for s in range(NS)]
    ot_t = [sb.tile([128, S], bf16, name=f"ott{f}", tag=f"ott{f}") for f in range(2)]
    bqk_t = sb.tile([128, 4], f32, name="bqkt", tag="bqkt")
    mask_t = sb.tile([128, 256], bf16, name="maskt", tag="maskt")
    ones_t = sb.tile([128, 1], bf16, name="ones1", tag="ones1")
    wu_t = sb.tile([128, 512], bf16, name="wut", tag="wut")
    nc.vector.memset(ones_t[:], 1.0)
    nc.vector.memset(wu_t[:], 0.0)

    # ---- input DMAs, ALL on the sync issue queue in strict priority order:
    # the 16 DMA ring engines serve descriptors in arrival order, so a single
    # ordered stream gives the critical prefix (wq, x cols 0:512) the full
    # HBM bandwidth instead of racing later tensors.
    xt3 = xt_a.rearrange("p (k s) -> p k s", k=NK)
    xs3 = xT.rearrange("(k p) s -> p k s", p=128)
    nc.sync.dma_start(wq_a.rearrange("p (k f) -> p k f", k=NK),
                      wq.rearrange("(k p) f -> p k f", p=128))
    nc.sync.dma_start(xt3[:, 0:4, 0:512], xs3[:, 0:4, 0:512])
    nc.sync.dma_start(xt3[:, 4:8, 0:512], xs3[:, 4:8, 0:512])
    nc.sync.dma_start(bqk_t[:], bqk[:])
    nc.sync.dma_start(mask_t[:], maskT[:])
    nc.sync.dma_start(wk_a.rearrange("p (k f) -> p k f", k=NK),
                      wk.rearrange("(k p) f -> p k f", p=128))
    nc.sync.dma_start(wv_a.rearrange("p (k f) -> p k f", k=NK),
                      wv.rearrange("(k p) f -> p k f", p=128))
    nc.sync.dma_start(xt3[:, :, 512:1024], xs3[:, :, 512:1024])
    nc.sync.dma_start(wp_a.rearrange("p (k f) -> p k f", k=2),
                      wp.rearrange("(k p) f -> p k f", p=128))
    nc.sync.dma_start(xt3[:, :, 1024:1536], xs3[:, :, 1024:1536])
    nc.sync.dma_start(xt3[:, :, 1536:2048], xs3[:, :, 1536:2048])

    # PSUM (8 banks): scores 2x[128,1024] (4), pv 2x[128,512] (2),
    # den 1x[128,512] (1), filler ring 1x[128,512] (1).
    scp = ctx.enter_context(tc.tile_pool(name="ps_sc", bufs=2, space="PSUM"))
    pvp = ctx.enter_context(tc.tile_pool(name="ps_pv", bufs=2, space="PSUM"))
    dnp = ctx.enter_context(tc.tile_pool(name="ps_dn", bufs=1, space="PSUM"))
    flp = ctx.enter_context(tc.tile_pool(name="ps_fl", bufs=1, space="PSUM"))

    p_pool = ctx.enter_context(tc.tile_pool(name="pp", bufs=1))
    rc_pool = ctx.enter_context(tc.tile_pool(name="rcp", bufs=8))
    oo_pool = ctx.enter_context(tc.tile_pool(name="oop", bufs=3))

    # ---- warmup: junk matmuls so HAM releases the PE clock gate before the
    # first real matmul; runs entirely during the DMA wait.
    wups = flp.tile([128, 512], f32, name="fl", tag="fl", bufs=1)
    for i in range(10):
        nc.tensor.matmul(wups[:], wu_t[:, 0:128], wu_t[:], start=True,
                         stop=True, skip_group_check=True)

    def qkt_half(dst, w_t, bcol, f, c, pool=None):
        """512-col chunk of the Q^T/K^T projection for feature split f.
        Prelude groups allocate from the (still idle) scores ring so they
        double-buffer; in-block fillers use the single filler bank."""
        tg = "sc" if pool is not None else "fl"
        ps = (pool or flp).tile([128, 512], f32, name=tg, tag=tg,
                                bufs=2 if pool is not None else 1)
        for k in range(NK):
            nc.tensor.matmul(
                ps[:],
                w_t[k][:, f * 128:(f + 1) * 128],
                xt_t[k][:, c * 512:(c + 1) * 512],
                start=(k == 0), stop=(k == NK - 1),
            )
        nc.vector.tensor_scalar_add(
            dst[f][:, c * 512:(c + 1) * 512], ps[:],
            bqk_t[:, bcol + f: bcol + f + 1],
        )

    def v_group(s, pool=None):
        tg = "sc" if pool is not None else "fl"
        psv = (pool or flp).tile([128, FPC], f32, name=tg, tag=tg,
                                 bufs=2 if pool is not None else 1)
        for k in range(NK):
            nc.tensor.matmul(
                psv[:],
                xt_t[k][:, s * 128:(s + 1) * 128],
                wv_t[k][:],
                start=(k == 0), stop=(k == NK - 1),
            )
        nc.vector.tensor_copy(v_t[s][:], psv[:])

    def proj_half(s, nh, act_drain=False, pool=None):
        """512 output columns of the projection for q rows [128s,128s+128)."""
        tg = "sc" if pool is not None else "fl"
        pj = (pool or flp).tile([128, 512], f32, name=tg, tag=tg,
                                bufs=2 if pool is not None else 1)
        for k2 in range(2):
            nc.tensor.matmul(
                pj[:],
                ot_t[k2][:, s * 128:(s + 1) * 128],
                wp_t[k2][:, nh * 512:(nh + 1) * 512],
                start=(k2 == 0), stop=(k2 == 1),
            )
        oo = oo_pool.tile([128, 512], bf16, name="oo", tag="oo", bufs=3)
        if act_drain:
            nc.scalar.copy(oo[:], pj[:])
        else:
            nc.vector.tensor_copy(oo[:], pj[:])
        nc.sync.dma_start(out[s * 128:(s + 1) * 128, nh * 512:(nh + 1) * 512],
                          oo[:])

    class Block:
        """Causal attention for all 4 heads over q cols [512b, 512b+512)."""

        def __init__(self, b):
            self.b = b
            self.n = 4 * b + 4          # number of ki blocks
            self.q0 = 512 * b
            self.pv = [pvp.tile([128, 512], f32, name="pv", tag="pv", bufs=2)
                       for _ in range(2)]
            self.dn = dnp.tile([128, 512], f32, name="dn", tag="dn", bufs=1)
            self.P = {}
            self.a0s = {}

        def a0(self, ki):
            return 128 * (ki - 4 * self.b) if ki >= 4 * self.b else 0

        def scores(self, ki):
            """Row-tiled pair per hp: heads 2hp (rows 0:64 -> cols 0:512 of
            sc) and 2hp+1 (rows 64:128 -> cols 512:1024)."""
            a0 = self.a0(ki)
            diag = ki >= 4 * self.b
            for hp in range(2):
                sc = scp.tile([128, 1024], f32, name="sc", tag="sc", bufs=2)
                qt, kt = qt_t[hp], kt_t[hp]
                for r0, off in ((0, 0), (64, 512)):
                    nc.tensor.matmul(
                        sc[:, off + a0:off + 512],
                        kt[r0:r0 + 64, ki * 128:(ki + 1) * 128],
                        qt[r0:r0 + 64, self.q0 + a0:self.q0 + 512],
                        start=True, stop=True,
                    )
                P = p_pool.tile([128, 1024], bf16, name="P", tag="P", bufs=8)
                if a0:
                    # one strided instr covering both heads' valid spans
                    sc3 = sc.rearrange("p (h c) -> p h c", h=2)
                    P3 = P.rearrange("p (h c) -> p h c", h=2)
                    nc.scalar.activation(P3[:, :, a0:512], sc3[:, :, a0:512],
                                         AF.Exp, scale=SCL)
                else:
                    nc.scalar.activation(P[:], sc[:], AF.Exp, scale=SCL)
                if diag:
                    P3 = P.rearrange("p (h c) -> p h c", h=2)
                    m3 = mask_t.rearrange("p (h c) -> p h c", h=2)
                    nc.vector.tensor_mul(P3[:, :, a0:a0 + 128],
                                         P3[:, :, a0:a0 + 128], m3[:])
                self.P[(hp, ki)] = P

        def pv_den(self, ki):
            """Col-tiled PV pair per hp + 4-wide den pass; 128-col chunk
            (ki-4b) stops at its diagonal ki."""
            a0 = self.a0(ki)
            diag = ki >= 4 * self.b
            spans = [(a0, a0 + 128, True)] if diag else []
            if diag and a0 + 128 < 512:
                spans.append((a0 + 128, 512, False))
            if not diag:
                spans = [(0, 512, False)]
            Ps = [self.P.pop((0, ki)), self.P.pop((1, ki))]
            for a, bnd, stops in spans:
                for hp in range(2):
                    P = Ps[hp]
                    for r0, off in ((0, 0), (64, 512)):
                        hcol = (2 * hp + (1 if r0 else 0)) * 64
                        nc.tensor.matmul(
                            self.pv[hp][r0:r0 + 64, a:bnd],
                            v_t[ki][:, hcol:hcol + 64],
                            P[:, off + a:off + bnd],
                            start=(ki == 0), stop=stops,
                        )
                for j in range(4):
                    P, off = Ps[j // 2], 512 * (j % 2)
                    nc.tensor.matmul(
                        self.dn[32 * j:32 * j + 1, a:bnd],
                        ones_t[:],
                        P[:, off + a:off + bnd],
                        start=(ki == 0), stop=stops,
                        tile_position=(0, 32 * j),
                    )

        def finish_a(self, a, w):
            """Cols [a,a+w) released: copy each denominator row out of psum
            (the custom reciprocal op misreads psum at partition base 32/96,
            plain copies do not — DVE proven at base 0/32, ACT at 64/96,
            which also splits the burst across two engines), reciprocal,
            gpsimd partition broadcast. finish_b consumes the rbc tiles one
            iteration later."""
            rbcs = []
            for h in range(4):
                dcp = rc_pool.tile([1, 512], f32, name="dc", tag="dc", bufs=8)
                if h < 2:
                    nc.vector.tensor_copy(
                        dcp[:, 0:w], self.dn[32 * h:32 * h + 1, a:a + w])
                else:
                    nc.scalar.copy(
                        dcp[:, 0:w], self.dn[32 * h:32 * h + 1, a:a + w])
                rcp = rc_pool.tile([1, 512], f32, name="rc", tag="rc", bufs=8)
                nc.vector.reciprocal_approx_fast(rcp[:, 0:w], dcp[:, 0:w])
                rbc = rc_pool.tile([64, 512], f32, name="rb", tag="rb", bufs=8)
                nc.gpsimd.partition_broadcast(rbc[:, 0:w], rcp[:, 0:w],
                                              channels=64)
                rbcs.append(rbc)
            return rbcs

        def finish_b(self, a, w, rbcs):
            for hp in range(2):
                for r0 in (0, 64):
                    h = 2 * hp + (1 if r0 else 0)
                    nc.vector.tensor_mul(
                        ot_t[hp][r0:r0 + 64, self.q0 + a:self.q0 + a + w],
                        self.pv[hp][r0:r0 + 64, a:a + w],
                        rbcs[h][:, 0:w])

    def warm_fill(n):
        """Keep-warm junk matmuls for thin drain iterations (sc ring is free
        once the last scores have issued)."""
        dum = scp.tile([128, 512], f32, name="sc", tag="sc", bufs=2)
        for _ in range(n):
            nc.tensor.matmul(dum[:], wu_t[:, 0:128], wu_t[:], start=True,
                             stop=True, skip_group_check=True)

    # ---- prelude: q/k projections for chunk 0, both feature splits. These
    # MUST be emitted before the first scores — the tile dep tracker links a
    # read only to writes emitted before it, so a later-emitted producer is
    # silently unordered (reads garbage). Prelude groups allocate from the
    # still-idle scores ring so consecutive groups double-buffer.
    qkt_half(qt_t, wq_t, 0, 0, 0, pool=scp)
    qkt_half(kt_t, wk_t, 2, 0, 0, pool=scp)
    qkt_half(qt_t, wq_t, 0, 1, 0, pool=scp)
    qkt_half(kt_t, wk_t, 2, 1, 0, pool=scp)

    # ---- flat global pipeline: the scores stream runs contiguously across
    # the four q blocks (no per-block drain bubbles); pv/den trail by LAG.
    # Block b's pool tiles are recycled exactly when its coarse finish (read)
    # precedes the next block's first accumulation (write) inside one
    # iteration. Fillers are placed per global iteration g.
    P = partial
    SL = [(b, ki) for b in range(4) for ki in range(4 * b + 4)]
    fillers = {
        0: [P(v_group, 0)],
        1: [P(v_group, 1)],
        2: [P(qkt_half, qt_t, wq_t, 0, 0, 1), P(v_group, 2)],
        3: [P(qkt_half, qt_t, wq_t, 0, 1, 1), P(v_group, 3)],
        4: [P(qkt_half, kt_t, wk_t, 2, 0, 1)],
        5: [P(qkt_half, kt_t, wk_t, 2, 1, 1)],
        6: [P(v_group, 4)],
        7: [P(v_group, 5)],
        8: [P(qkt_half, qt_t, wq_t, 0, 0, 2), P(v_group, 6)],
        9: [P(qkt_half, qt_t, wq_t, 0, 1, 2), P(v_group, 7)],
        10: [P(qkt_half, kt_t, wk_t, 2, 0, 2)],
        11: [P(qkt_half, kt_t, wk_t, 2, 1, 2)],
        12: [P(v_group, 8)],
        13: [P(v_group, 9)],
        14: [P(v_group, 10)],
        15: [P(v_group, 11)],
        16: [P(qkt_half, qt_t, wq_t, 0, 0, 3)],
        17: [P(qkt_half, qt_t, wq_t, 0, 1, 3)],
        18: [P(proj_half, 0, 0)],
        19: [P(proj_half, 0, 1)],
        20: [P(proj_half, 1, 0)],
        21: [P(proj_half, 1, 1)],
        22: [P(proj_half, 2, 0)],
        23: [P(proj_half, 2, 1)],
        24: [P(v_group, 12)],
        25: [P(v_group, 13)],
        26: [P(v_group, 14)],
        27: [P(v_group, 15)],
        28: [P(proj_half, 3, 0)],
        29: [P(proj_half, 3, 1)],
        30: [P(qkt_half, kt_t, wk_t, 2, 0, 3)],
        31: [P(qkt_half, kt_t, wk_t, 2, 1, 3)],
        32: [P(proj_half, 4, 0)],
        33: [P(proj_half, 4, 1)],
        34: [P(proj_half, 5, 0)],
        35: [P(proj_half, 5, 1)],
        36: [P(proj_half, 6, 0), P(proj_half, 6, 1)],
        37: [P(proj_half, 7, 0), P(proj_half, 7, 1)],
        38: [P(proj_half, 8, 0, True), P(proj_half, 8, 1, False)],
        39: [P(proj_half, 9, 0, True), P(proj_half, 9, 1, False)],
        40: [P(proj_half, 10, 0, True, scp), P(proj_half, 10, 1, False, scp),
             P(proj_half, 12, 0, True, scp), P(proj_half, 12, 1, False, scp),
             P(warm_fill, 6)],
        41: [P(proj_half, 11, 0, True, scp), P(proj_half, 11, 1, False, scp),
             P(proj_half, 13, 0, True, scp), P(proj_half, 13, 1, False, scp),
             P(warm_fill, 6)],
    }
    blocks = {}
    pend = []
    for g in range(len(SL) + LAG):
        for fb in pend:
            fb()
        pend = []
        if g < len(SL):
            b, ki = SL[g]
            if ki == 0:
                blocks[b] = Block(b)
            blocks[b].scores(ki)
        if g >= LAG:
            b, kj = SL[g - LAG]
            blk = blocks[b]
            blk.pv_den(kj)
            if b == 3 and kj >= 12:
                a = 128 * (kj - 12)
                pend.append(
                    partial(blk.finish_b, a, 128, blk.finish_a(a, 128)))
            elif b < 3 and kj in (4 * b + 1, 4 * b + 3):
                a = 0 if kj == 4 * b + 1 else 256
                pend.append(
                    partial(blk.finish_b, a, 256, blk.finish_a(a, 256)))
        for fn in fillers.get(g, []):
            fn()
    warm_fill(3)
    for fb in pend:
        fb()
    proj_half(14, 0, True, pool=scp)
    proj_half(14, 1, False, pool=scp)
    warm_fill(3)
    proj_half(15, 0, True, pool=scp)
    proj_half(15, 1, False, pool=scp)


def _in_maps(x, W_qkv, b_qkv, W_proj):
    bf = ml_dtypes.bfloat16
    maps = []
    # multiplicative causal mask for the transposed diag block: keep k<=q,
    # two copies side by side so one strided multiply covers a head pair
    mask1 = np.triu(np.ones((128, 128), np.float32))
    mask = np.concatenate([mask1, mask1], axis=1).astype(bf)
    for core in range(NCORES):
        b, hg = core // 4, core % 4
        cs = slice(hg * FPC, (hg + 1) * FPC)
        bq = b_qkv[cs].astype(np.float32)
        bk = b_qkv[D + hg * FPC: D + (hg + 1) * FPC].astype(np.float32)
        maps.append({
            "xT": np.ascontiguousarray(x[b].T).astype(bf),
            "wq": np.ascontiguousarray(W_qkv[:, cs]).astype(bf),
            "wk": np.ascontiguousarray(W_qkv[:, D + hg * FPC: D + (hg + 1) * FPC]).astype(bf),
            "wv": np.ascontiguousarray(W_qkv[:, 2 * D + hg * FPC: 2 * D + (hg + 1) * FPC]).astype(bf),
            "wp": np.ascontiguousarray(W_proj[hg * FPC:(hg + 1) * FPC, :]).astype(bf),
            "bqk": np.ascontiguousarray(
                np.stack([bq[0:128], bq[128:256], bk[0:128], bk[128:256]], axis=1)),
            "maskT": mask,
        })
    return maps


def get_nc():
    if "nc" not in _CACHE:
        _CACHE["nc"] = _build()
    return _CACHE["nc"]


def _postprocess(partials, b_qkv, W_proj, b_proj):
    out = np.zeros((B, S, D), np.float32)
    for core in range(NCORES):
        out[core // 4] += np.asarray(partials[core], np.float32)
    bv = np.asarray(b_qkv, np.float32)[2 * D:3 * D]
    out += bv @ np.asarray(W_proj, np.float32) + np.asarray(b_proj, np.float32)
    return out


def kernel(x, W_qkv, b_qkv, W_proj, b_proj, _trace=False):
    from concourse.bass_utils import run_bass_kernel_spmd

    x = np.asarray(x, np.float32)
    W_qkv = np.asarray(W_qkv, np.float32)
    b_qkv = np.asarray(b_qkv, np.float32)
    W_proj = np.asarray(W_proj, np.float32)
    b_proj = np.asarray(b_proj, np.float32)

    nc = get_nc()
    maps = _in_maps(x, W_qkv, b_qkv, W_proj)
    res = run_bass_kernel_spmd(nc, maps, list(range(NCORES)), trace=_trace)
    _CACHE["last_result"] = res
    partials = [res.results[c]["out"] for c in range(NCORES)]
    return _postprocess(partials, b_qkv, W_proj, b_proj)


# revision 41
# speedup vs baseline: 1.1633x; 1.0379x over previous
"""Causal multi-head attention (B=2, S=2048, D=1024, H=16, hd=64) on 8 trn2 cores.

Sharding: core c handles batch b = c//4 and head group hg = c%4 (4 heads each).
Each core computes its Q/K/V shard (tensor-parallel columns of W_qkv), causal
attention for its 4 heads with scores held transposed ([s_k, s_q]), and a
partial output projection over its 256 rows of W_proj. The host sums the 4
partials per batch and adds the exact bias terms (softmax rows sum to 1, so
attn@(V + 1 bv^T) = attn@V + bv^T; the b_qkv V-slice and b_proj are applied on
the host).

PE packing (the key win over the naive schedule): the systolic array is
addressed in 32-row/32-col tiles, so matmuls on disjoint row/col groups run
concurrently when adjacent in the queue:
  - scores: K=64 per head -> the two heads of a pair run row-tiled at
    tile positions (0,0)/(64,0), one [128,1024] psum tile (A|B layout, one
    bank each), ~217ns per 512-q-col pair instead of 2x216.
  - PV: M=64 per head -> col-tiled pair at (0,0)/(0,64) into one [128,512]
    psum tile (head A partitions 0-63, head B 64-127), separate moving
    operands (the A|B halves of the P tile).
  - softmax denominators: 4 heads' ones-row matmuls (M=1) col-tiled at
    (0,{0,32,64,96}) into one psum bank -> all 4 denominators per 512 cols
    in ~227ns (replaces the 65th-V-column trick, freeing PV for col tiling).
Pairs share one psum tile so both MMs have identical WAR deps and the
priority scheduler keeps them adjacent (separated pairs don't overlap).

Attention runs on 512-col q blocks (4 per core), all 4 heads per iteration
over ki; pv/den lag scores by 2 iterations to hide the exp latency. Each
128-col q chunk stops accumulating at its diagonal ki, so finish (reciprocal
-> gpsimd broadcast -> scale, two-phase) and the output projection stagger
through the whole kernel instead of trailing it.

DMA: all input DMAs ride the sync issue queue in strict priority order (wq,
x cols 0:512, bqk, mask, wk, wv, x 512:1024, wp, rest of x); the 16 DMA ring
engines drain descriptors in arrival order, so issue order ~= completion
order and the critical prefix gets full HBM bandwidth. Junk warmup matmuls
run during the DMA wait (and in the thin drain iterations at the end) so the
PE's HAM clock gate stays released while real work is pending.
"""

import numpy as np
import ml_dtypes
from contextlib import ExitStack

B, S, D, H = 2, 2048, 1024, 16
HD = 64
NCORES = 8
FPC = 256  # features per core (4 heads x 64)

_CACHE = {}


def _build():
    import concourse.bacc as bacc
    import concourse.tile as tile
    import concourse.mybir as mybir

    f32 = mybir.dt.float32
    bf16 = mybir.dt.bfloat16

    nc = bacc.Bacc("TRN2", target_bir_lowering=False, debug=False, num_devices=NCORES)

    xT = nc.dram_tensor("xT", [D, S], bf16, kind="ExternalInput").ap()
    wq = nc.dram_tensor("wq", [D, FPC], bf16, kind="ExternalInput").ap()
    wk = nc.dram_tensor("wk", [D, FPC], bf16, kind="ExternalInput").ap()
    wv = nc.dram_tensor("wv", [D, FPC], bf16, kind="ExternalInput").ap()
    wp = nc.dram_tensor("wp", [FPC, D], bf16, kind="ExternalInput").ap()
    bqk = nc.dram_tensor("bqk", [128, 4], f32, kind="ExternalInput").ap()
    maskT = nc.dram_tensor("maskT", [128, 256], bf16, kind="ExternalInput").ap()
    out = nc.dram_tensor("out", [S, D], bf16, kind="ExternalOutput").ap()

    with tile.TileContext(nc) as tc:
        with ExitStack() as ctx:
            _body(ctx, tc, mybir, out, xT, wq, wk, wv, wp, bqk, maskT)

    nc.compile()
    return nc


def _body(ctx, tc, mybir, out, xT, wq, wk, wv, wp, bqk, maskT):
    from functools import partial

    nc = tc.nc
    f32 = mybir.dt.float32
    bf16 = mybir.dt.bfloat16
    AF = mybir.ActivationFunctionType
    NK = D // 128   # 8 contraction tiles for qkv/proj-input dim
    NS = S // 128   # 16 sequence (key) tiles
    LAG = 2         # iterations between scores and pv/den
    SCL = float(HD) ** -0.5

    sb = ctx.enter_context(tc.tile_pool(name="sb", bufs=1))

    xt_a = sb.tile([128, NK * S], bf16, name="xta", tag="xta")
    xt_t = [xt_a[:, k * S:(k + 1) * S] for k in range(NK)]
    wq_a = sb.tile([128, NK * FPC], bf16, name="wqa", tag="wqa")
    wq_t = [wq_a[:, k * FPC:(k + 1) * FPC] for k in range(NK)]
    wk_a = sb.tile([128, NK * FPC], bf16, name="wka", tag="wka")
    wk_t = [wk_a[:, k * FPC:(k + 1) * FPC] for k in range(NK)]
    wv_a = sb.tile([128, NK * FPC], bf16, name="wva", tag="wva")
    wv_t = [wv_a[:, k * FPC:(k + 1) * FPC] for k in range(NK)]
    wp_a = sb.tile([128, 2 * D], bf16, name="wpa", tag="wpa")
    wp_t = [wp_a[:, k * D:(k + 1) * D] for k in range(2)]
    qt_t = [sb.tile([128, S], bf16, name=f"qtt{f}", tag=f"qtt{f}") for f in range(2)]
    kt_t = [sb.tile([128, S], bf16, name=f"ktt{f}", tag=f"ktt{f}") for f in range(2)]
    v_t = [sb.tile([128, FPC], bf16, name=f"vt{s}", tag=f"vt{s}") for s in range(NS)]
    ot_t = [sb.tile([128, S], bf16, name=f"ott{f}", tag=f"ott{f}") for f in range(2)]
    bqk_t = sb.tile([128, 4], f32, name="bqkt", tag="bqkt")
    mask_t = sb.tile([128, 256], bf16, name="maskt", tag="maskt")
    ones_t = sb.tile([128, 1], bf16, name="ones1", tag="ones1")
    wu_t = sb.tile([128, 512], bf16, name="wut", tag="wut")
    nc.vector.memset(ones_t[:], 1.0)
    nc.vector.memset(wu_t[:], 0.0)

    # ---- input DMAs, ALL on the sync issue queue in strict priority order:
    # the 16 DMA ring engines serve descriptors in arrival order, so a single
    # ordered stream gives the critical prefix (wq, x cols 0:512) the full
    # HBM bandwidth instead of racing later tensors.
    xt3 = xt_a.rearrange("p (k s) -> p k s", k=NK)
    xs3 = xT.rearrange("(k p) s -> p k s", p=128)
    nc.sync.dma_start(wq_a.rearrange("p (k f) -> p k f", k=NK),
                      wq.rearrange("(k p) f -> p k f", p=128))
    nc.sync.dma_start(xt3[:, 0:4, 0:512], xs3[:, 0:4, 0:512])
    nc.sync.dma_start(xt3[:, 4:8, 0:512], xs3[:, 4:8, 0:512])
    nc.sync.dma_start(bqk_t[:], bqk[:])
    nc.sync.dma_start(mask_t[:], maskT[:])
    nc.sync.dma_start(wk_a.rearrange("p (k f) -> p k f", k=NK),
                      wk.rearrange("(k p) f -> p k f", p=128))
    nc.sync.dma_start(wv_a.rearrange("p (k f) -> p k f", k=NK),
                      wv.rearrange("(k p) f -> p k f", p=128))
    nc.sync.dma_start(xt3[:, :, 512:1024], xs3[:, :, 512:1024])
    nc.sync.dma_start(wp_a.rearrange("p (k f) -> p k f", k=2),
                      wp.rearrange("(k p) f -> p k f", p=128))
    nc.sync.dma_start(xt3[:, :, 1024:1536], xs3[:, :, 1024:1536])
    nc.sync.dma_start(xt3[:, :, 1536:2048], xs3[:, :, 1536:2048])

    # PSUM (8 banks): scores 2x[128,1024] (4), pv 2x[128,512] (2),
    # den 1x[128,512] (1), filler ring 1x[128,512] (1).
    scp = ctx.enter_context(tc.tile_pool(name="ps_sc", bufs=2, space="PSUM"))
    pvp = ctx.enter_context(tc.tile_pool(name="ps_pv", bufs=2, space="PSUM"))
    dnp = ctx.enter_context(tc.tile_pool(name="ps_dn", bufs=1, space="PSUM"))
    flp = ctx.enter_context(tc.tile_pool(name="ps_fl", bufs=1, space="PSUM"))

    p_pool = ctx.enter_context(tc.tile_pool(name="pp", bufs=1))
    rc_pool = ctx.enter_context(tc.tile_pool(name="rcp", bufs=8))
    oo_pool = ctx.enter_context(tc.tile_pool(name="oop", bufs=3))

    # ---- warmup: junk matmuls so HAM releases the PE clock gate before the
    # first real matmul; runs entirely during the DMA wait.
    wups = flp.tile([128, 512], f32, name="fl", tag="fl", bufs=1)
    for i in range(16):
        nc.tensor.matmul(wups[:], wu_t[:, 0:128], wu_t[:], start=True,
                         stop=True, skip_group_check=True)

    def qkt_half(dst, w_t, bcol, f, c, pool=None):
        """512-col chunk of the Q^T/K^T projection for feature split f.
        Prelude groups allocate from the (still idle) scores ring so they
        double-buffer; in-block fillers use the single filler bank."""
        tg = "sc" if pool is not None else "fl"
        ps = (pool or flp).tile([128, 512], f32, name=tg, tag=tg,
                                bufs=2 if pool is not None else 1)
        for k in range(NK):
            nc.tensor.matmul(
                ps[:],
                w_t[k][:, f * 128:(f + 1) * 128],
                xt_t[k][:, c * 512:(c + 1) * 512],
                start=(k == 0), stop=(k == NK - 1),
            )
        nc.vector.tensor_scalar_add(
            dst[f][:, c * 512:(c + 1) * 512], ps[:],
            bqk_t[:, bcol + f: bcol + f + 1],
        )

    def v_group(s, pool=None):
        tg = "sc" if pool is not None else "fl"
        psv = (pool or flp).tile([128, FPC], f32, name=tg, tag=tg,
                                 bufs=2 if pool is not None else 1)
        for k in range(NK):
            nc.tensor.matmul(
                psv[:],
                xt_t[k][:, s * 128:(s + 1) * 128],
                wv_t[k][:],
                start=(k == 0), stop=(k == NK - 1),
            )
        nc.vector.tensor_copy(v_t[s][:], psv[:])

    def proj_half(s, nh, act_drain=False, pool=None):
        """512 output columns of the projection for q rows [128s,128s+128)."""
        tg = "sc" if pool is not None else "fl"
        pj = (pool or flp).tile([128, 512], f32, name=tg, tag=tg,
                                bufs=2 if pool is not None else 1)
        for k2 in range(2):
            nc.tensor.matmul(
                pj[:],
                ot_t[k2][:, s * 128:(s + 1) * 128],
                wp_t[k2][:, nh * 512:(nh + 1) * 512],
                start=(k2 == 0), stop=(k2 == 1),
            )
        oo = oo_pool.tile([128, 512], bf16, name="oo", tag="oo", bufs=3)
        if act_drain:
            nc.scalar.copy(oo[:], pj[:])
        else:
            nc.vector.tensor_copy(oo[:], pj[:])
        nc.sync.dma_start(out[s * 128:(s + 1) * 128, nh * 512:(nh + 1) * 512],
                          oo[:])

    class Block:
        """Causal attention for all 4 heads over q cols [512b, 512b+512)."""

        def __init__(self, b):
            self.b = b
            self.n = 4 * b + 4          # number of ki blocks
            self.q0 = 512 * b
            self.pv = [pvp.tile([128, 512], f32, name="pv", tag="pv", bufs=2)
                       for _ in range(2)]
            self.dn = dnp.tile([128, 512], f32, name="dn", tag="dn", bufs=1)
            self.P = {}
            self.a0s = {}

        def a0(self, ki):
            return 128 * (ki - 4 * self.b) if ki >= 4 * self.b else 0

        def scores(self, ki):
            """Row-tiled pair per hp: heads 2hp (rows 0:64 -> cols 0:512 of
            sc) and 2hp+1 (rows 64:128 -> cols 512:1024)."""
            a0 = self.a0(ki)
            diag = ki >= 4 * self.b
            for hp in range(2):
                sc = scp.tile([128, 1024], f32, name="sc", tag="sc", bufs=2)
                qt, kt = qt_t[hp], kt_t[hp]
                for r0, off in ((0, 0), (64, 512)):
                    nc.tensor.matmul(
                        sc[:, off + a0:off + 512],
                        kt[r0:r0 + 64, ki * 128:(ki + 1) * 128],
                        qt[r0:r0 + 64, self.q0 + a0:self.q0 + 512],
                        start=True, stop=True,
                    )
                P = p_pool.tile([128, 1024], bf16, name="P", tag="P", bufs=8)
                if a0:
                    # one strided instr covering both heads' valid spans
                    sc3 = sc.rearrange("p (h c) -> p h c", h=2)
                    P3 = P.rearrange("p (h c) -> p h c", h=2)
                    nc.scalar.activation(P3[:, :, a0:512], sc3[:, :, a0:512],
                                         AF.Exp, scale=SCL)
                else:
                    nc.scalar.activation(P[:], sc[:], AF.Exp, scale=SCL)
                if diag:
                    P3 = P.rearrange("p (h c) -> p h c", h=2)
                    m3 = mask_t.rearrange("p (h c) -> p h c", h=2)
                    nc.vector.tensor_mul(P3[:, :, a0:a0 + 128],
                                         P3[:, :, a0:a0 + 128], m3[:])
                self.P[(hp, ki)] = P

        def pv_den(self, ki):
            """Col-tiled PV pair per hp + 4-wide den pass; 128-col chunk
            (ki-4b) stops at its diagonal ki."""
            a0 = self.a0(ki)
            diag = ki >= 4 * self.b
            spans = [(a0, a0 + 128, True)] if diag else []
            if diag and a0 + 128 < 512:
                spans.append((a0 + 128, 512, False))
            if not diag:
                spans = [(0, 512, False)]
            Ps = [self.P.pop((0, ki)), self.P.pop((1, ki))]
            for a, bnd, stops in spans:
                for hp in range(2):
                    P = Ps[hp]
                    for r0, off in ((0, 0), (64, 512)):
                        hcol = (2 * hp + (1 if r0 else 0)) * 64
                        nc.tensor.matmul(
                            self.pv[hp][r0:r0 + 64, a:bnd],
                            v_t[ki][:, hcol:hcol + 64],
                            P[:, off + a:off + bnd],
                            start=(ki == 0), stop=stops,
                        )
                for j in range(4):
                    P, off = Ps[j // 2], 512 * (j % 2)
                    nc.tensor.matmul(
                        self.dn[32 * j:32 * j + 1, a:bnd],
                        ones_t[:],
                        P[:, off + a:off + bnd],
                        start=(ki == 0), stop=stops,
                        tile_position=(0, 32 * j),
                    )

        def finish_a(self, a, w):
            """Cols [a,a+w) released: copy each denominator row out of psum
            (the custom reciprocal op misreads psum at partition base 32/96,
            plain copies do not — DVE proven at base 0/32, ACT at 64/96,
            which also splits the burst across two engines), reciprocal,
            gpsimd partition broadcast. finish_b consumes the rbc tiles one
            iteration later."""
            rbcs = []
            for h in range(4):
                dcp = rc_pool.tile([1, 512], f32, name="dc", tag="dc", bufs=8)
                if h < 2:
                    nc.vector.tensor_copy(
                        dcp[:, 0:w], self.dn[32 * h:32 * h + 1, a:a + w])
                else:
                    nc.scalar.copy(
                        dcp[:, 0:w], self.dn[32 * h:32 * h + 1, a:a + w])
                rcp = rc_pool.tile([1, 512], f32, name="rc", tag="rc", bufs=8)
                nc.vector.reciprocal_approx_fast(rcp[:, 0:w], dcp[:, 0:w])
                rbc = rc_pool.tile([64, 512], f32, name="rb", tag="rb", bufs=8)
                nc.gpsimd.partition_broadcast(rbc[:, 0:w], rcp[:, 0:w],
                                              channels=64)
                rbcs.append(rbc)
            return rbcs

        def finish_b(self, a, w, rbcs):
            for hp in range(2):
                for r0 in (0, 64):
                    h = 2 * hp + (1 if r0 else 0)
                    nc.vector.tensor_mul(
                        ot_t[hp][r0:r0 + 64, self.q0 + a:self.q0 + a + w],
                        self.pv[hp][r0:r0 + 64, a:a + w],
                        rbcs[h][:, 0:w])

    def warm_fill(n):
        """Keep-warm junk matmuls for thin drain iterations (sc ring is free
        once the last scores have issued)."""
        dum = scp.tile([128, 512], f32, name="sc", tag="sc", bufs=2)
        for _ in range(n):
            nc.tensor.matmul(dum[:], wu_t[:, 0:128], wu_t[:], start=True,
                             stop=True, skip_group_check=True)

    # ---- prelude: q/k projections for chunk 0, both feature splits. These
    # MUST be emitted before the first scores — the tile dep tracker links a
    # read only to writes emitted before it, so a later-emitted producer is
    # silently unordered (reads garbage). Prelude groups allocate from the
    # still-idle scores ring so consecutive groups double-buffer.
    qkt_half(qt_t, wq_t, 0, 0, 0, pool=scp)
    qkt_half(kt_t, wk_t, 2, 0, 0, pool=scp)
    qkt_half(qt_t, wq_t, 0, 1, 0, pool=scp)
    qkt_half(kt_t, wk_t, 2, 1, 0, pool=scp)

    # ---- flat global pipeline: the scores stream runs contiguously across
    # the four q blocks (no per-block drain bubbles); pv/den trail by LAG.
    # Block b's pool tiles are recycled exactly when its coarse finish (read)
    # precedes the next block's first accumulation (write) inside one
    # iteration. Fillers are placed per global iteration g.
    P = partial
    SL = [(b, ki) for b in range(4) for ki in range(4 * b + 4)]
    fillers = {
        0: [P(v_group, 0)],
        1: [P(v_group, 1)],
        2: [P(qkt_half, qt_t, wq_t, 0, 0, 1), P(v_group, 2)],
        3: [P(qkt_half, qt_t, wq_t, 0, 1, 1), P(v_group, 3)],
        4: [P(qkt_half, kt_t, wk_t, 2, 0, 1)],
        5: [P(qkt_half, kt_t, wk_t, 2, 1, 1)],
        6: [P(v_group, 4)],
        7: [P(v_group, 5)],
        8: [P(qkt_half, qt_t, wq_t, 0, 0, 2), P(v_group, 6)],
        9: [P(qkt_half, qt_t, wq_t, 0, 1, 2), P(v_group, 7)],
        10: [P(qkt_half, kt_t, wk_t, 2, 0, 2)],
        11: [P(qkt_half, kt_t, wk_t, 2, 1, 2)],
        12: [P(v_group, 8)],
        13: [P(v_group, 9)],
        14: [P(v_group, 10)],
        15: [P(v_group, 11)],
        16: [P(qkt_half, qt_t, wq_t, 0, 0, 3)],
        17: [P(qkt_half, qt_t, wq_t, 0, 1, 3)],
        18: [P(proj_half, 0, 0)],
        19: [P(proj_half, 0, 1)],
        20: [P(proj_half, 1, 0)],
        21: [P(proj_half, 1, 1)],
        22: [P(proj_half, 2, 0)],
        23: [P(proj_half, 2, 1)],
        24: [P(v_group, 12)],
        25: [P(v_group, 13)],
        26: [P(v_group, 14)],
        27: [P(v_group, 15)],
        28: [P(proj_half, 3, 0)],
        29: [P(proj_half, 3, 1)],
        30: [P(qkt_half, kt_t, wk_t, 2, 0, 3)],
        31: [P(qkt_half, kt_t, wk_t, 2, 1, 3)],
        32: [P(proj_half, 4, 0)],
        33: [P(proj_half, 4, 1)],
        34: [P(proj_half, 5, 0)],
        35: [P(proj_half, 5, 1)],
        36: [P(proj_half, 6, 0), P(proj_half, 6, 1)],
        37: [P(proj_half, 7, 0), P(proj_half, 7, 1)],
        38: [P(proj_half, 8, 0, True), P(proj_half, 8, 1, False)],
        39: [P(proj_half, 9, 0, True), P(proj_half, 9, 1, False)],
        40: [P(proj_half, 10, 0, True, scp), P(proj_half, 10, 1, False, scp),
             P(proj_half, 12, 0, True, scp), P(proj_half, 12, 1, False, scp),
             P(warm_fill, 6)],
        41: [P(proj_half, 11, 0, True, scp), P(proj_half, 11, 1, False, scp),
             P(proj_half, 13, 0, True, scp), P(proj_half, 13, 1, False, scp),
             P(warm_fill, 6)],
    }
    blocks = {}
    pend = []
    for g in range(len(SL) + LAG):
        for fb in pend:
            fb()
        pend = []
        if g < len(SL):
            b, ki = SL[g]
            if ki == 0:
                blocks[b] = Block(b)
            blocks[b].scores(ki)
        if g >= LAG:
            b, kj = SL[g - LAG]
            blk = blocks[b]
            blk.pv_den(kj)
            if b == 3 and kj >= 12:
                a = 128 * (kj - 12)
                pend.append(
                    partial(blk.finish_b, a, 128, blk.finish_a(a, 128)))
            elif b < 3 and kj >= 4 * b + 1:
                # (0,256) @ diag+1, (256,384) @ diag+2, (384,512) @ diag+3:
                # the final part is only 128 cols, so the next block's first
                # pv accumulation (WAR on these reads) clears quickly.
                a, w = {1: (0, 256), 2: (256, 128), 3: (384, 128)}[kj - 4 * b]
                pend.append(
                    partial(blk.finish_b, a, w, blk.finish_a(a, w)))
        for fn in fillers.get(g, []):
            fn()
    warm_fill(3)
    for fb in pend:
        fb()
    proj_half(14, 0, True, pool=scp)
    proj_half(14, 1, False, pool=scp)
    warm_fill(3)
    proj_half(15, 0, True, pool=scp)
    proj_half(15, 1, False, pool=scp)


def _in_maps(x, W_qkv, b_qkv, W_proj):
    bf = ml_dtypes.bfloat16
    maps = []
    # multiplicative causal mask for the transposed diag block: keep k<=q,
    # two copies side by side so one strided multiply covers a head pair
    mask1 = np.triu(np.ones((128, 128), np.float32))
    mask = np.concatenate([mask1, mask1], axis=1).astype(bf)
    for core in range(NCORES):
        b, hg = core // 4, core % 4
        cs = slice(hg * FPC, (hg + 1) * FPC)
        bq = b_qkv[cs].astype(np.float32)
        bk = b_qkv[D + hg * FPC: D + (hg + 1) * FPC].astype(np.float32)
        maps.append({
            "xT": np.ascontiguousarray(x[b].T).astype(bf),
            "wq": np.ascontiguousarray(W_qkv[:, cs]).astype(bf),
            "wk": np.ascontiguousarray(W_qkv[:, D + hg * FPC: D + (hg + 1) * FPC]).astype(bf),
            "wv": np.ascontiguousarray(W_qkv[:, 2 * D + hg * FPC: 2 * D + (hg + 1) * FPC]).astype(bf),
            "wp": np.ascontiguousarray(W_proj[hg * FPC:(hg + 1) * FPC, :]).astype(bf),
            "bqk": np.ascontiguousarray(
                np.stack([bq[0:128], bq[128:256], bk[0:128], bk[128:256]], axis=1)),
            "maskT": mask,
        })
    return maps


def get_nc():
    if "nc" not in _CACHE:
        _CACHE["nc"] = _build()
    return _CACHE["nc"]


def _postprocess(partials, b_qkv, W_proj, b_proj):
    out = np.zeros((B, S, D), np.float32)
    for core in range(NCORES):
        out[core // 4] += np.asarray(partials[core], np.float32)
    bv = np.asarray(b_qkv, np.float32)[2 * D:3 * D]
    out += bv @ np.asarray(W_proj, np.float32) + np.asarray(b_proj, np.float32)
    return out


def kernel(x, W_qkv, b_qkv, W_proj, b_proj, _trace=False):
    from concourse.bass_utils import run_bass_kernel_spmd

    x = np.asarray(x, np.float32)
    W_qkv = np.asarray(W_qkv, np.float32)
    b_qkv = np.asarray(b_qkv, np.float32)
    W_proj = np.asarray(W_proj, np.float32)
    b_proj = np.asarray(b_proj, np.float32)

    nc = get_nc()
    maps = _in_maps(x, W_qkv, b_qkv, W_proj)
    res = run_bass_kernel_spmd(nc, maps, list(range(NCORES)), trace=_trace)
    _CACHE["last_result"] = res
    partials = [res.results[c]["out"] for c in range(NCORES)]
    return _postprocess(partials, b_qkv, W_proj, b_proj)
